# revision 13
# baseline (speedup 1.0000x reference)
"""Trainium2 Bass kernel for nn_AnomalyBlock (dense transformer block with
gaussian prior + causal attention), distributed over 8 NeuronCores.

Sharding:
  - Attention: head-parallel. 16 (batch, head) pairs -> 2 pairs per core
    (core c: batch c//4, heads 2*(c%4), 2*(c%4)+1). Full causal structure per
    pair => identical SPMD graph, perfectly balanced.
  - Output projection is a partial sum over local heads; ReduceScatter(add)
    over each 4-core group yields row-sharded out1; LN/FFN chain is
    position-wise and runs on the local 512-row slice.
  - sigma/prior: row-parallel (core c: rows [c*256,(c+1)*256) of both batches).
  - series upper triangle is never written (outputs are donated zero buffers).

Precision:
  - Matmuls in float32r (full PE speed, ~1e-3) for q/k/v/scores/out/lin1.
  - sigma via fp16 3-term split (~6e-6, fp32-class) because prior's
    exp(-d^2/2sigma) amplifies sigma error by the exponent magnitude (~88).
  - softmax: exp on ACT (bf16 out for fast PE transposes, fp32 row sums via
    accum), normalization in fp32. Masked scores get -1e30 pre-exp => exact 0.
"""
import math
import os
import numpy as np

import concourse.bass as bass
import concourse.bacc as bacc
import concourse.mybir as mybir
import concourse.tile as tile
from concourse import masks
from concourse.bass_utils import run_bass_kernel_spmd

B, S, E, H = 2, 2048, 512, 8
W = S
D = E // H
P = 128
NKC = E // P          # 4 K-chunks of the E contraction
NST = S // P          # 16 seq tiles of 128
NCC = S // 512        # 4 column chunks of 512
ROWS_SIG = S // 8     # 256 sigma rows per core
F32 = mybir.dt.float32
F32R = mybir.dt.float32r
F16 = mybir.dt.float16
BF16 = mybir.dt.bfloat16
EXP = mybir.ActivationFunctionType.Exp
COPY = mybir.ActivationFunctionType.Copy
SQRT = mybir.ActivationFunctionType.Sqrt
ADD = mybir.AluOpType.add
SUB = mybir.AluOpType.subtract
MULT = mybir.AluOpType.mult

_CACHED = {}


def _layernorm(nc, pool, t1, out_ap, ones512, eps):
    """LN over E=512 free axis with only HW-proven op classes."""
    if os.environ.get("K_SIMPLE_LN"):
        nc.vector.tensor_copy(out_ap, t1)
        return
    s1 = pool.tile([P, 1], F32, tag="ln_s1", name="ln_s1")
    nc.vector.tensor_reduce(out=s1[:], in_=t1, axis=mybir.AxisListType.X, op=ADD)
    m = pool.tile([P, 1], F32, tag="ln_m", name="ln_m")
    nc.scalar.activation(m[:], s1[:], COPY, scale=1.0 / E)
    cent = pool.tile([P, E], F32, tag="ln_cent", name="ln_cent")
    nc.vector.scalar_tensor_tensor(
        out=cent[:], in0=t1, scalar=m[:], in1=ones512[:], op0=SUB, op1=MULT)
    sq = pool.tile([P, E], F32, tag="ln_sq", name="ln_sq")
    ve = pool.tile([P, 1], F32, tag="ln_ve", name="ln_ve")
    nc.vector.tensor_tensor(out=sq[:], in0=cent[:], in1=cent[:], op=MULT)
    nc.vector.tensor_reduce(out=ve[:], in_=sq[:], axis=mybir.AxisListType.X, op=ADD)
    sd = pool.tile([P, 1], F32, tag="ln_sd", name="ln_sd")
    nc.scalar.activation(sd[:], ve[:], SQRT, scale=1.0 / E, bias=eps[:])
    rstd = pool.tile([P, 1], F32, tag="ln_rstd", name="ln_rstd")
    nc.vector.reciprocal(rstd[:], sd[:])
    nc.vector.scalar_tensor_tensor(
        out=out_ap, in0=cent[:], scalar=rstd[:], in1=ones512[:], op0=MULT, op1=MULT)


def build():
    nc = bacc.Bacc("TRN2", target_bir_lowering=False, debug=False, num_devices=8)

    def din(name, shape, dt=F32):
        return nc.dram_tensor(name, shape, dt, kind="ExternalInput")

    def dout(name, shape, dt=F32):
        return nc.dram_tensor(name, shape, dt, kind="ExternalOutput")

    xTb = din("xTb", [E, S], F32R)            # x[bc].T  (this core's batch)
    wqts = din("wqts", [E, P], F32R)          # Wq.T cols for 2 local heads
    wkts = din("wkts", [E, P], F32R)
    wvts = din("wvts", [E, 256], F32R)        # Wv.T local heads in 65-stride pad
    wots = din("wots", [P, E], F32R)          # Wo.T rows for 2 local heads
    lin1t = din("lin1t", [E, E], F32R)
    bqs = din("bqs", [1, P], F32R)
    bks = din("bks", [1, P], F32R)
    bvs = din("bvs", [1, 256], F32R)
    bo4 = din("bo4", [1, E], F32R)            # bo / 4 (summed over quad by RS)
    lin1b = din("lin1b", [1, E], F32R)
    xr_ffn = din("xr_ffn", [512, E])          # x rows for post-RS residual
    xtr_h = din("xtr_h", [B, E, ROWS_SIG], F16)   # xT cols at sigma rows, hi
    xtr_l = din("xtr_l", [B, E, ROWS_SIG], F16)
    wst_h = din("wst_h", [E, W], F16)
    wst_l = din("wst_l", [E, W], F16)
    bsg_h = din("bsg_h", [1, W], F16)
    bsg_l = din("bsg_l", [1, W], F16)
    dist2r = din("dist2r", [2, P, W])         # (r-c)^2 rows per local sig tile
    ones_in = din("ones_in", [1, 2048], F32R)

    series_sh = dout("series_sh", [2, S, S])      # per local (b,h) pair
    prior_sh = dout("prior_sh", [B, 2, P, W])
    out_sh = dout("out_sh", [512, E])

    INV_SQRT_2PI = float(1.0 / math.sqrt(2.0 * math.pi))

    with tile.TileContext(nc) as tc:
        with (
            tc.tile_pool(name="const", bufs=1) as cpool,
            tc.tile_pool(name="big", bufs=1) as bigp,
            tc.tile_pool(name="small", bufs=3) as sp,
            tc.tile_pool(name="psA", bufs=2, space="PSUM") as psA,
            tc.tile_pool(name="psB", bufs=2, space="PSUM") as psB,
            tc.tile_pool(name="psC", bufs=2, space="PSUM") as psC,
            tc.tile_pool(name="dram", bufs=1, space="DRAM") as dpool,
        ):
            # ---------------- persistent constants (~27KB/part) -----------
            def ld(pool, dram_t, shape, dt, rearr=None, name=None, **kw):
                t = pool.tile(shape, dt, name=name, tag=name or "")
                src = dram_t.ap()
                if rearr:
                    src = src.rearrange(rearr, **kw)
                nc.sync.dma_start(t[:], src)
                return t

            wq_sb = ld(cpool, wqts, [P, NKC, P], F32R, "(kc p) n -> p kc n", p=P, name="wq_sb")
            wk_sb = ld(cpool, wkts, [P, NKC, P], F32R, "(kc p) n -> p kc n", p=P, name="wk_sb")
            wv_sb = ld(cpool, wvts, [P, NKC, 256], F32R, "(kc p) n -> p kc n", p=P, name="wv_sb")
            wo_sb = ld(cpool, wots, [P, E], F32R, name="wo_sb")
            l1_sb = ld(cpool, lin1t, [P, NKC, E], F32R, "(kc p) n -> p kc n", p=P, name="l1_sb")
            bq_sb = ld(cpool, bqs, [1, P], F32R, name="bq_sb")
            bk_sb = ld(cpool, bks, [1, P], F32R, name="bk_sb")
            bv_sb = ld(cpool, bvs, [1, 256], F32R, name="bv_sb")
            bo_sb = ld(cpool, bo4, [1, E], F32R, name="bo_sb")
            l1b_sb = ld(cpool, lin1b, [1, E], F32R, name="l1b_sb")

            ones_r = ld(cpool, ones_in, [1, 2048], F32R, name="ones_r")
            ones_h = cpool.tile([1, P], F16, name="ones_h")
            nc.gpsimd.memset(ones_h[:], 1.0)
            ident16 = cpool.tile([P, P], BF16, name="ident16")
            masks.make_identity(nc, ident16[:])
            ident32 = cpool.tile([P, P], F32, name="ident32")
            masks.make_identity(nc, ident32[:])
            # emask: [:, :128] causal (0 on/below diag, -1e30 above), rest -1e30
            emask = cpool.tile([P, 640], F32, name="emask")
            nc.gpsimd.memset(emask[:], -1e30)
            nc.gpsimd.memset(emask[:, 0:P], 0.0)
            nc.gpsimd.affine_select(
                out=emask[:, 0:P], in_=emask[:, 0:P],
                compare_op=mybir.AluOpType.is_ge, fill=-1e30,
                base=0, pattern=[[-1, P]], channel_multiplier=1)

            # ---------------- sigma / prior (row-sharded) ----------------
            if os.environ.get("K_SKIP_SIGMA"):
                sigma_on = False
            else:
                sigma_on = True
            with (
                tc.tile_pool(name="sigc", bufs=1) as sigc,
                tc.tile_pool(name="sigw", bufs=2) as sigw,
            ):
                xh_sb = ld(sigc, xtr_h, [P, B, NKC, ROWS_SIG], F16,
                           "b (kc p) n -> p b kc n", p=P, name="xh_sb")
                xl_sb = ld(sigc, xtr_l, [P, B, NKC, ROWS_SIG], F16,
                           "b (kc p) n -> p b kc n", p=P, name="xl_sb")
                bsh_sb = ld(sigc, bsg_h, [1, W], F16, name="bsh_sb")
                bsl_sb = ld(sigc, bsg_l, [1, W], F16, name="bsl_sb")
                for wc in range(NCC if sigma_on else 0):
                    cw = slice(wc * 512, (wc + 1) * 512)
                    wsh_c = sigw.tile([P, NKC, 512], F16, name="wsh_c")
                    nc.sync.dma_start(
                        wsh_c[:], wst_h.ap()[:, cw].rearrange("(kc p) n -> p kc n", p=P))
                    wsl_c = sigw.tile([P, NKC, 512], F16, name="wsl_c")
                    nc.sync.dma_start(
                        wsl_c[:], wst_l.ap()[:, cw].rearrange("(kc p) n -> p kc n", p=P))
                    for lt in range(2):
                        d2c = sigw.tile([P, 512], F32, name="d2c")
                        nc.sync.dma_start(d2c[:], dist2r.ap()[lt, :, cw])
                        for b in range(B):
                            ps = psA.tile([P, 512], F32, tag="mm512", name="ps_sig")
                            n = 0
                            for kc in range(NKC):
                                for (At, Bt) in ((xh_sb, wsh_c), (xh_sb, wsl_c), (xl_sb, wsh_c)):
                                    nc.tensor.matmul(
                                        ps[:], At[:, b, kc, lt * P:(lt + 1) * P],
                                        Bt[:, kc, :], start=(n == 0), stop=False)
                                    n += 1
                            nc.tensor.matmul(ps[:], ones_h[:], bsh_sb[:, cw],
                                             start=False, stop=False)
                            nc.tensor.matmul(ps[:], ones_h[:], bsl_sb[:, cw],
                                             start=False, stop=True)
                            rs = sigw.tile([P, 512], F32, name="rs")
                            nc.vector.reciprocal(rs[:], ps[:])
                            u = sigw.tile([P, 512], F32, name="u")
                            nc.vector.tensor_tensor(out=u[:], in0=d2c[:], in1=rs[:], op=MULT)
                            pe = sigw.tile([P, 512], F32, name="pe")
                            nc.scalar.activation(pe[:], u[:], EXP, scale=-0.5)
                            pr = sigw.tile([P, 512], F32, name="pr")
                            nc.vector.scalar_tensor_tensor(
                                out=pr[:], in0=pe[:], scalar=INV_SQRT_2PI, in1=rs[:],
                                op0=MULT, op1=MULT)
                            nc.sync.dma_start(prior_sh.ap()[b, lt, :, cw], pr[:])

            # ---------------- projections (local batch) ----------------
            kt_sb = bigp.tile([P, S], F32R, name="kt_sb")   # [2*64 hd, S]
            qt_sb = bigp.tile([P, S], F32R, name="qt_sb")
            v_sb = bigp.tile([P, NST, 130], BF16, name="v_sb")
            attnT_sb = bigp.tile([P, S], F32R, name="attnT_sb")

            with tc.tile_pool(name="xtp", bufs=1) as xtp:
                xt_sb = xtp.tile([P, NKC, S], F32R, name="xt_sb")
                nc.sync.dma_start(xt_sb[:], xTb.ap().rearrange("(kc p) n -> p kc n", p=P))
                for cc in range(NCC):
                    for (wmat, bmat, dst) in ((wk_sb, bk_sb, kt_sb), (wq_sb, bq_sb, qt_sb)):
                        ps = psA.tile([P, 512], F32, tag="mm512", name="ps_kq")
                        for kc in range(NKC):
                            nc.tensor.matmul(ps[:], wmat[:, kc, :],
                                             xt_sb[:, kc, cc * 512:(cc + 1) * 512],
                                             start=(kc == 0), stop=False)
                        nc.tensor.matmul(ps[:], bmat[:], ones_r[:, 0:512],
                                         start=False, stop=True)
                        nc.vector.tensor_copy(dst[:, cc * 512:(cc + 1) * 512], ps[:])
                for st in range(NST):
                    ps = psB.tile([P, 256], F32, tag="psB", name="ps_v")
                    for kc in range(NKC):
                        nc.tensor.matmul(ps[:], xt_sb[:, kc, st * P:(st + 1) * P],
                                         wv_sb[:, kc, :], start=(kc == 0), stop=False)
                    nc.tensor.matmul(ps[:], ones_r[:, 0:P], bv_sb[:],
                                     start=False, stop=True)
                    nc.scalar.activation(v_sb[:, st, 0:130], ps[:, 0:130], COPY)
                vv = v_sb[:].rearrange("p st (ph x) -> p st ph x", x=65)
                nc.gpsimd.memset(vv[:, :, :, 64:65], 1.0)

            # ---------------- attention (2 local pairs) ----------------
            with tc.tile_pool(name="attw", bufs=2) as aw:
                for ph in range(0 if os.environ.get("K_SKIP_ATTN") else 2):
                    hp = ph * 64
                    for rg in range(4):
                        nch = rg + 1
                        pex = {}
                        for tt in range(4):
                            t = rg * 4 + tt
                            pex[tt] = aw.tile([P, 2048], BF16, tag=f"pex{tt}",
                                              name=f"pex{tt}")
                            sums = sp.tile([P, NCC], F32, tag="sums", name="sums")
                            for cc in range(nch):
                                ps = psA.tile([P, 512], F32, tag="mm512", name="ps_sc")
                                nc.tensor.matmul(
                                    ps[:], qt_sb[hp:hp + 64, t * P:(t + 1) * P],
                                    kt_sb[hp:hp + 64, cc * 512:(cc + 1) * 512],
                                    start=True, stop=True)
                                if cc == rg:
                                    off = (t % 4) * P
                                    nc.vector.tensor_tensor(
                                        out=ps[:, off:512], in0=ps[:, off:512],
                                        in1=emask[:, 0:512 - off], op=ADD)
                                nc.scalar.activation(
                                    pex[tt][:, cc * 512:(cc + 1) * 512], ps[:], EXP,
                                    scale=0.125, accum_out=sums[:, cc:cc + 1])
                            tot = sp.tile([P, 1], F32, tag="tot", name="tot")
                            if nch > 1:
                                nc.vector.tensor_reduce(
                                    out=tot[:], in_=sums[:, 0:nch],
                                    axis=mybir.AxisListType.X, op=ADD)
                            else:
                                nc.vector.tensor_copy(tot[:], sums[:, 0:1])
                            rec = sp.tile([P, 1], F32, tag="rec", name="rec")
                            nc.vector.reciprocal(rec[:], tot[:])
                            for cc in range(nch):
                                wlast = (t % 4 + 1) * P if cc == rg else 512
                                nrm = aw.tile([P, 512], F32, tag="nrm", name="nrm", bufs=3)
                                nc.scalar.activation(
                                    nrm[:, 0:wlast], pex[tt][:, cc * 512:cc * 512 + wlast],
                                    COPY, scale=rec[:])
                                nc.sync.dma_start(
                                    series_sh.ap()[ph, t * P:(t + 1) * P,
                                                   cc * 512:cc * 512 + wlast],
                                    nrm[:, 0:wlast])
                        # AV over rows rg*512..(rg+1)*512
                        pat = psC.tile([65, 512], F32, tag="attnT", name="pat")
                        for c1 in range(nch * 4):
                            ptp = psB.tile([P, 512], BF16, tag="psB", name="ptp")
                            for tt in range(4):
                                nc.tensor.transpose(
                                    ptp[:, tt * P:(tt + 1) * P],
                                    pex[tt][:, c1 * P:(c1 + 1) * P], ident16[:])
                            pts = aw.tile([P, 512], BF16, tag="pts", name="pts", bufs=3)
                            nc.vector.tensor_copy(pts[:], ptp[:])
                            nc.tensor.matmul(
                                pat[:], v_sb[:, c1, ph * 65:ph * 65 + 65], pts[:],
                                start=(c1 == 0), stop=(c1 == nch * 4 - 1))
                        # normalize attn^T rows by 1/rowsum (row sums = pat[64])
                        rrow = sp.tile([1, 512], F32R, tag="rrow", name="rrow")
                        with nc.allow_low_precision(reason="f32r for matmul rhs"):
                            nc.vector.reciprocal(rrow[:], pat[64:65, :])
                        bc = psC.tile([64, 512], F32, tag="bcast", name="bc", bufs=1)
                        nc.tensor.matmul(bc[:], ones_r[:, 0:64], rrow[:],
                                         start=True, stop=True)
                        pat_sb = sp.tile([64, 512], F32, tag="pat_sb", name="pat_sb")
                        nc.vector.tensor_copy(pat_sb[:], pat[0:64, :])
                        with nc.allow_low_precision(reason="f32r for matmul lhsT"):
                            nc.vector.tensor_tensor(
                                out=attnT_sb[hp:hp + 64, rg * 512:(rg + 1) * 512],
                                in0=pat_sb[:], in1=bc[:], op=MULT)

                # ---------------- out projection partials -> RS -----------
                rs_in = dpool.tile([S, E], F32, name="rs_in")
                rs_out = dpool.tile([512, E], F32, name="rs_out")
                for st in range(0 if os.environ.get("K_SKIP_FFN") else NST):
                    ps = psA.tile([P, 512], F32, tag="mm512", name="ps_o1")
                    nc.tensor.matmul(ps[:], attnT_sb[:, st * P:(st + 1) * P],
                                     wo_sb[:], start=True, stop=False)
                    nc.tensor.matmul(ps[:], ones_r[:, 0:P], bo_sb[:],
                                     start=False, stop=True)
                    o1 = aw.tile([P, E], F32, tag="o1", name="o1")
                    nc.scalar.activation(o1[:], ps[:], COPY)
                    nc.sync.dma_start(rs_in[st * P:(st + 1) * P, :], o1[:])
                if not os.environ.get("K_SKIP_FFN"):
                    nc.gpsimd.collective_compute(
                        "ReduceScatter", ADD,
                        replica_groups=[[0, 1, 2, 3], [4, 5, 6, 7]],
                        ins=[rs_in[:].opt()], outs=[rs_out[:].opt()])

            # ---------------- LN1 -> lin1 -> LN2 (512 local rows) ----------
            with tc.tile_pool(name="ffnp", bufs=1) as fp, \
                 tc.tile_pool(name="ffnw", bufs=2) as fw:
                if os.environ.get("K_SKIP_FFN"):
                    NKC_F = 0
                else:
                    NKC_F = NKC
                xr_sb = ld(fp, xr_ffn, [P, NKC, E], F32, "(st p) n -> p st n", p=P,
                           name="xr_sb")
                ones512 = fp.tile([P, E], F32, name="ones512")
                nc.gpsimd.memset(ones512[:], 1.0)
                eps_t = fp.tile([P, 1], F32, name="eps_t")
                nc.gpsimd.memset(eps_t[:], 1e-5)
                ln1_sb = fp.tile([P, NKC, E], F32, name="ln1_sb")
                l1T_sb = fp.tile([P, NKC, 512], F32R, name="l1T_sb")
                for st in range(NKC_F):
                    t1 = fw.tile([P, E], F32, tag="t1", name="t1")
                    rsw = fw.tile([P, E], F32, tag="rsw", name="rsw")
                    nc.sync.dma_start(rsw[:], rs_out[st * P:(st + 1) * P, :])
                    nc.vector.tensor_tensor(out=t1[:], in0=rsw[:], in1=xr_sb[:, st, :],
                                            op=ADD)
                    _layernorm(nc, fw, t1[:], ln1_sb[:, st, :], ones512, eps_t)
                for st in range(NKC_F):
                    for kc in range(NKC):
                        pt = psB.tile([P, P], F32, tag="psB", name="pt")
                        nc.tensor.transpose(pt[:], ln1_sb[:, st, kc * P:(kc + 1) * P],
                                            ident32[:])
                        with nc.allow_low_precision(reason="f32r for matmul lhsT"):
                            nc.vector.tensor_copy(l1T_sb[:, kc, st * P:(st + 1) * P], pt[:])
                for st in range(NKC_F):
                    ps = psA.tile([P, 512], F32, tag="mm512", name="ps_l1")
                    for kc in range(NKC):
                        nc.tensor.matmul(
                            ps[:], l1T_sb[:, kc, st * P:(st + 1) * P],
                            l1_sb[:, kc, :], start=(kc == 0), stop=False)
                    nc.tensor.matmul(ps[:], ones_r[:, 0:P], l1b_sb[:],
                                     start=False, stop=True)
                    t2 = fw.tile([P, E], F32, tag="t2", name="t2")
                    nc.vector.tensor_tensor(out=t2[:], in0=ps[:], in1=ln1_sb[:, st, :],
                                            op=ADD)
                    o2 = fw.tile([P, E], F32, tag="o2", name="o2")
                    _layernorm(nc, fw, t2[:], o2[:], ones512, eps_t)
                    nc.sync.dma_start(out_sh.ap()[st * P:(st + 1) * P, :], o2[:])

    nc.compile()
    return nc


def kernel(x, Wq, bq, Wk, bk, Wv, bv, Wsig, bsig, Wo, bo,
           lin1_W, lin1_b, ln1_w, ln1_b, ln2_w, ln2_b):
    x = np.asarray(x, np.float32)
    assert np.allclose(np.asarray(ln1_w), 1.0) and np.allclose(np.asarray(ln1_b), 0.0)
    assert np.allclose(np.asarray(ln2_w), 1.0) and np.allclose(np.asarray(ln2_b), 0.0)

    if "nc" not in _CACHED:
        _CACHED["nc"] = build()
    nc = _CACHED["nc"]

    xT = np.ascontiguousarray(x.transpose(0, 2, 1))            # [B, E, S]
    WqT = np.ascontiguousarray(np.asarray(Wq, np.float32).T)   # [E, E] in,out
    WkT = np.ascontiguousarray(np.asarray(Wk, np.float32).T)
    WvT = np.ascontiguousarray(np.asarray(Wv, np.float32).T)
    WoT = np.ascontiguousarray(np.asarray(Wo, np.float32).T)
    lin1T = np.ascontiguousarray(np.asarray(lin1_W, np.float32).T)
    WsT = np.ascontiguousarray(np.asarray(Wsig, np.float32).T)  # [E, W]
    WsT_h = WsT.astype(np.float16)
    WsT_l = (WsT - WsT_h.astype(np.float32)).astype(np.float16)
    bsig = np.asarray(bsig, np.float32)
    bsg_h = bsig.astype(np.float16)
    bsg_l = (bsig - bsg_h.astype(np.float32)).astype(np.float16)
    bq = np.asarray(bq, np.float32); bk = np.asarray(bk, np.float32)
    bv = np.asarray(bv, np.float32); bo = np.asarray(bo, np.float32)
    lin1_b = np.asarray(lin1_b, np.float32)
    i_idx = np.arange(S, dtype=np.float32)
    dist2 = (i_idx[:, None] - i_idx[None, :]) ** 2              # [S, S] f32

    in_maps = []
    for c in range(8):
        bc, q = c // 4, c % 4
        hA = 2 * q
        hd = slice(hA * D, (hA + 2) * D)                        # 128 head dims
        wv_p = np.zeros((E, 256), np.float32)
        wv_p[:, 0:64] = WvT[:, hA * D:(hA + 1) * D]
        wv_p[:, 65:129] = WvT[:, (hA + 1) * D:(hA + 2) * D]
        bv_p = np.zeros((1, 256), np.float32)
        bv_p[0, 0:64] = bv[hA * D:(hA + 1) * D]
        bv_p[0, 65:129] = bv[(hA + 1) * D:(hA + 2) * D]
        rsg = slice(c * ROWS_SIG, (c + 1) * ROWS_SIG)
        xtr = np.ascontiguousarray(xT[:, :, rsg])               # [B, E, 256]
        xtr_h = xtr.astype(np.float16)
        xtr_l = (xtr - xtr_h.astype(np.float32)).astype(np.float16)
        d2r = np.stack([dist2[c * ROWS_SIG:c * ROWS_SIG + P],
                        dist2[c * ROWS_SIG + P:(c + 1) * ROWS_SIG]])
        in_maps.append({
            "xTb": xT[bc], "wqts": np.ascontiguousarray(WqT[:, hd]),
            "wkts": np.ascontiguousarray(WkT[:, hd]),
            "wvts": wv_p, "wots": np.ascontiguousarray(WoT[hd, :]), "lin1t": lin1T,
            "bqs": np.ascontiguousarray(bq[None, hd]),
            "bks": np.ascontiguousarray(bk[None, hd]), "bvs": bv_p,
            "bo4": bo[None, :] / 4.0, "lin1b": np.ascontiguousarray(lin1_b[None, :]),
            "xr_ffn": np.ascontiguousarray(x[bc, q * 512:(q + 1) * 512, :]),
            "xtr_h": xtr_h, "xtr_l": xtr_l,
            "wst_h": WsT_h, "wst_l": WsT_l,
            "bsg_h": np.ascontiguousarray(bsg_h[None, :]),
            "bsg_l": np.ascontiguousarray(bsg_l[None, :]),
            "dist2r": d2r,
            "ones_in": np.ones((1, 2048), np.float32),
        })

    res = run_bass_kernel_spmd(nc, in_maps, list(range(8)))

    out = np.zeros((B, S, E), np.float32)
    series = np.zeros((B, H, S, S), np.float32)
    prior = np.zeros((B, S, W), np.float32)
    for c in range(8):
        r = res.results[c]
        bc, q = c // 4, c % 4
        hA = 2 * q
        series[bc, hA] = r["series_sh"][0]
        series[bc, hA + 1] = r["series_sh"][1]
        out[bc, q * 512:(q + 1) * 512] = r["out_sh"]
        prior[:, c * ROWS_SIG:c * ROWS_SIG + P] = r["prior_sh"][:, 0]
        prior[:, c * ROWS_SIG + P:(c + 1) * ROWS_SIG] = r["prior_sh"][:, 1]
    return out, series, prior
